# revision 5
# baseline (speedup 1.0000x reference)
"""Trainium2 Bass kernel for CantorAttention.

Strategy
--------
The Cantor routes are a pure function of the (quantized) Cantor value of each
position: sorting positions by that value makes every query's 64-key route set
live inside a narrow (<=385-wide) window of the sorted order.  Sparse
attention therefore becomes dense *banded* attention after a host-side
permutation:

  host:   pi = argsort(cantor_val), permute x rows, transpose; build per
          128-query-tile 128-aligned windows of width 384 plus an additive
          bf16 mask (-30000 at non-selected slots).
  device: qkvT projection (fp32r matmuls), banded scores + mask (PE),
          exp+rowsum (ACT, fused accum), normalize (DVE), PE-transpose of the
          probabilities into per-128-chunk column-major buffers, PV matmuls
          accumulating transposed attention output, and the output projection
          producing a partial (4-head) outT block.
  host:   sum the 4 partial outT blocks per batch, transpose, un-permute,
          add the output bias.

Sharding: batch x head-block -> 8 cores (core c: b = c//4, heads 4*(c%4)..).
"""

import sys

sys.path.insert(0, "/opt/trn_rl_repo")

import numpy as np

B, S, DIM = 2, 2048, 1024
HEADS, DH = 16, 64
K_NEI = 64
N_CORES = 8
HPC = 4            # heads per core
QT = 128           # query tile (rows per tile)
NT = S // QT       # 16 query tiles
SUP = 4            # query tiles per supertile (PV batch of 512 queries)
NSUP = NT // SUP

_CACHE = {}


def _cantor_val(seq_len, depth=8):
    pos = np.arange(seq_len, dtype=np.float64)
    x = pos / max(1, seq_len - 1)
    x = np.clip(x, 1e-6, 1.0 - 1e-6)
    val = np.zeros_like(x)
    factor = 0.5
    for _ in range(depth):
        xs = x * 3.0
        digit = np.floor(xs)
        x = xs - digit
        val = val + (digit == 2.0).astype(np.float64) * factor
        factor *= 0.5
    return np.clip(val, 0.0, 1.0)


def _geometry(routes):
    """Window geometry from the runtime routes array."""
    val = _cantor_val(S)
    pi = np.argsort(val, kind="stable").astype(np.int64)
    rank = np.empty(S, np.int64)
    rank[pi] = np.arange(S)
    kr = rank[np.asarray(routes, np.int64)][pi]      # [S, K] key ranks, query-rank order
    lo = kr.min(1)
    hi = kr.max(1) + 1
    for win in (384, 512):
        a = np.zeros(NT, np.int64)
        ok = True
        for t in range(NT):
            l = int(lo[t * QT:(t + 1) * QT].min())
            h = int(hi[t * QT:(t + 1) * QT].max())
            a[t] = min(l // 128, (S - win) // 128)
            if h > a[t] * 128 + win:
                ok = False
                break
        if ok:
            return pi, rank, kr, a, win
    raise ValueError("routes structure incompatible with banded-window kernel")


def _build_module(a, win):
    from concourse import bacc, tile, mybir
    from concourse.masks import make_identity

    f32 = mybir.dt.float32
    f32r = mybir.dt.float32r
    bf16 = mybir.dt.bfloat16
    AF = mybir.ActivationFunctionType
    NCH = win // 128                      # chunks per window
    a = [int(v) for v in a]

    # chunk -> [first tile, last tile] using it
    chunk_tiles = {}
    for t in range(NT):
        for j in range(NCH):
            c = a[t] + j
            lo_t, hi_t = chunk_tiles.get(c, (t, t))
            chunk_tiles[c] = (min(lo_t, t), max(hi_t, t))

    nc = bacc.Bacc("TRN2", target_bir_lowering=False, debug=False)
    xT = nc.dram_tensor("xT", [DIM, S], f32r, kind="ExternalInput").ap()
    wq = nc.dram_tensor("wq", [DIM, 3 * HPC * DH], f32r, kind="ExternalInput").ap()
    bq = nc.dram_tensor("bq", [3 * HPC * DH, 1], f32, kind="ExternalInput").ap()
    wo = nc.dram_tensor("wo", [HPC * DH, DIM], f32r, kind="ExternalInput").ap()
    mask = nc.dram_tensor("mask", [QT, NT * win], bf16, kind="ExternalInput").ap()
    outp = nc.dram_tensor("outp", [DIM, S], f32, kind="ExternalOutput").ap()

    NQKV = 3 * HPC * DH                  # 768 rows of qkvT
    NMT = NQKV // 128                    # 6 row-tiles of qkvT

    with tile.TileContext(nc) as tc:
        with tc.tile_pool(name="persist", bufs=1) as pp:
            # persistent tiles
            id32 = pp.tile([128, 128], f32)
            make_identity(nc, id32)
            id_r = pp.tile([128, 128], f32r)
            nc.vector.tensor_copy(id_r, id32)
            id_b = pp.tile([128, 128], bf16)
            nc.vector.tensor_copy(id_b, id32)
            mask_sb = pp.tile([QT, NT * win], bf16)
            nc.sync.dma_start(out=mask_sb, in_=mask)
            bq_sb = []
            for m in range(NMT):
                bt = pp.tile([128, 1], f32, tag=f"bq{m}")
                nc.sync.dma_start(out=bt, in_=bq[m * 128:(m + 1) * 128, :])
                bq_sb.append(bt)
            qkvT = [pp.tile([128, S], f32r, tag=f"qkvT{m}", name=f"qkvT{m}") for m in range(NMT)]
            attn_outT = [pp.tile([128, S], f32r, tag=f"aout{p2}", name=f"aout{p2}") for p2 in range(2)]
            wo_sb = []
            for p2 in range(2):
                wt = pp.tile([128, DIM], f32r, tag=f"wo{p2}")
                nc.sync.dma_start(out=wt, in_=wo[p2 * 128:(p2 + 1) * 128, :])
                wo_sb.append(wt)

            # ---------------- Phase A: qkvT = wq.T @ xT (+bias) ----------------
            with tc.tile_pool(name="phA", bufs=1) as pa, \
                 tc.tile_pool(name="phAx", bufs=2) as pax, \
                 tc.tile_pool(name="psA", bufs=3, space="PSUM") as psa:
                wq_sb = []
                for kk in range(8):
                    wt = pa.tile([128, NQKV], f32r, tag=f"wq{kk}")
                    nc.sync.dma_start(out=wt, in_=wq[kk * 128:(kk + 1) * 128, :])
                    wq_sb.append(wt)
                eng_flip = 0
                for n in range(4):                    # 512-wide column chunks
                    xt = []
                    for kk in range(8):
                        t_ = pax.tile([128, 512], f32r, tag=f"x{kk}")
                        nc.sync.dma_start(
                            out=t_, in_=xT[kk * 128:(kk + 1) * 128, n * 512:(n + 1) * 512])
                        xt.append(t_)
                    for m in range(NMT):
                        ps = psa.tile([128, 512], f32, tag="ps")
                        for kk in range(8):
                            nc.tensor.matmul(
                                ps, wq_sb[kk][:, m * 128:(m + 1) * 128], xt[kk],
                                start=(kk == 0), stop=(kk == 7))
                        dst = qkvT[m][:, n * 512:(n + 1) * 512]
                        if eng_flip % 2 == 0:
                            nc.scalar.activation(out=dst, in_=ps, func=AF.Identity,
                                                 bias=bq_sb[m])
                        else:
                            nc.vector.tensor_scalar_add(dst, ps, bq_sb[m])
                        eng_flip += 1

            # ---------------- Phases B-D ----------------
            with tc.tile_pool(name="phC", bufs=1) as pc, \
                 tc.tile_pool(name="pexp_pool", bufs=3) as pe_pool, \
                 tc.tile_pool(name="pt_pool", bufs=10) as pt_pool, \
                 tc.tile_pool(name="small", bufs=8) as sm_pool, \
                 tc.tile_pool(name="psB", bufs=4, space="PSUM") as psb, \
                 tc.tile_pool(name="psS", bufs=2, space="PSUM") as pss, \
                 tc.tile_pool(name="psO", bufs=2, space="PSUM") as pso:
                # Phase B: V_all = vT.T per 128-chunk  (V_sb[cc][:, s*128:] )
                V_sb = [pc.tile([128, 2 * 128], f32r, tag=f"V{cc}", name=f"V{cc}") for cc in range(NT)]
                for cc in range(NT):
                    for s_ in range(2):
                        pv = psb.tile([128, 128], f32r, tag="ptr")
                        nc.tensor.transpose(
                            pv, qkvT[4 + s_][:, cc * 128:(cc + 1) * 128], id_r)
                        dst = V_sb[cc][:, s_ * 128:(s_ + 1) * 128]
                        if (cc + s_) % 2 == 0:
                            nc.vector.tensor_copy(dst, pv)
                        else:
                            nc.scalar.copy(dst, pv)

                # Phase C: banded attention per head
                aoutB = [pc.tile([64, S], f32r, tag=f"aoutB{i}", name=f"aoutB{i}")
                         for i in range(2)]
                for h in range(HPC):
                    poff = (h % 2) * 64
                    qTh = qkvT[h // 2]
                    kTh = qkvT[2 + h // 2]
                    pt_tiles = {}
                    for u in range(NSUP):
                        for t in range(u * SUP, (u + 1) * SUP):
                            w0 = a[t] * 128
                            ps_s = pss.tile([128, win], f32, tag="sc")
                            nc.tensor.matmul(
                                ps_s,
                                qTh[poff:poff + 64, t * 128:(t + 1) * 128],
                                kTh[poff:poff + 64, w0:w0 + win],
                                start=True, stop=False, skip_group_check=True)
                            nc.tensor.matmul(
                                ps_s, id_b, mask_sb[:, t * win:(t + 1) * win],
                                start=False, stop=True, skip_group_check=True)
                            pexp = pe_pool.tile([128, win], f32, tag="pexp")
                            denom = sm_pool.tile([128, 1], f32, tag="den")
                            nc.scalar.activation(out=pexp, in_=ps_s, func=AF.Exp,
                                                 accum_out=denom)
                            recip = sm_pool.tile([128, 1], f32, tag="rec")
                            nc.vector.reciprocal(recip, denom)
                            pnorm = pe_pool.tile([128, win], f32r, tag="pnorm")
                            nc.vector.tensor_scalar_mul(pnorm, pexp, recip)
                            for j in range(NCH):
                                c = a[t] + j
                                t0c, t1c = chunk_tiles[c]
                                if c not in pt_tiles:
                                    pt_tiles[c] = pt_pool.tile(
                                        [128, (t1c - t0c + 1) * 128], f32r, tag="pt", name=f"pt_h{h}_c{c}")
                                ptp = psb.tile([128, 128], f32r, tag="ptr")
                                nc.tensor.transpose(
                                    ptp, pnorm[:, j * 128:(j + 1) * 128], id_r)
                                dst = pt_tiles[c][:, (t - t0c) * 128:(t - t0c + 1) * 128]
                                if j % 2 == 0:
                                    nc.scalar.copy(dst, ptp)
                                else:
                                    nc.vector.tensor_copy(dst, ptp)
                        # PV for supertile u
                        chunks_u = sorted({a[t] + j
                                           for t in range(u * SUP, (u + 1) * SUP)
                                           for j in range(NCH)})
                        po = pso.tile([128, 512], f32, tag="po")
                        nc.vector.memset(po[0:64, :], 0.0)
                        for i_c, c in enumerate(chunks_u):
                            t0c, t1c = chunk_tiles[c]
                            tlo = max(t0c, u * SUP)
                            thi = min(t1c, (u + 1) * SUP - 1)
                            rhs = pt_tiles[c][:, (tlo - t0c) * 128:(thi - t0c + 1) * 128]
                            ocol0 = tlo * 128 - u * 512
                            ocol1 = (thi + 1) * 128 - u * 512
                            nc.tensor.matmul(
                                po[0:64, ocol0:ocol1],
                                V_sb[c][:, h * 64:(h + 1) * 64],
                                rhs,
                                start=False, stop=(i_c == len(chunks_u) - 1),
                                skip_group_check=True)
                        if poff == 0:
                            dst = attn_outT[h // 2][0:64, u * 512:(u + 1) * 512]
                        else:
                            dst = aoutB[h // 2][:, u * 512:(u + 1) * 512]
                        if (h + u) % 2 == 0:
                            nc.vector.tensor_copy(dst, po[0:64, :])
                        else:
                            nc.scalar.copy(dst, po[0:64, :])
                    if poff != 0:
                        nc.sync.dma_start(out=attn_outT[h // 2][64:128, :],
                                          in_=aoutB[h // 2])

                # Phase D: outp = wo.T @ attn_outT
                with tc.tile_pool(name="phD", bufs=4) as pd:
                    for mm in range(8):
                        for n in range(4):
                            ps = pso.tile([128, 512], f32, tag="po")
                            for p2 in range(2):
                                nc.tensor.matmul(
                                    ps, wo_sb[p2][:, mm * 128:(mm + 1) * 128],
                                    attn_outT[p2][:, n * 512:(n + 1) * 512],
                                    start=(p2 == 0), stop=(p2 == 1))
                            st = pd.tile([128, 512], f32, tag="st")
                            if (mm + n) % 2 == 0:
                                nc.scalar.copy(st, ps)
                            else:
                                nc.vector.tensor_copy(st, ps)
                            nc.sync.dma_start(
                                out=outp[mm * 128:(mm + 1) * 128, n * 512:(n + 1) * 512],
                                in_=st)

    nc.compile()
    return nc


def _get_module(a, win):
    key = (tuple(int(v) for v in a), int(win))
    if key not in _CACHE:
        _CACHE[key] = _build_module(a, win)
    return _CACHE[key]


def kernel(x, routes, qkv_w, qkv_b, out_w, out_b):
    import ml_dtypes
    from concourse.bass_utils import run_bass_kernel_spmd

    x = np.ascontiguousarray(np.asarray(x, np.float32))
    routes = np.asarray(routes)
    qkv_w = np.asarray(qkv_w, np.float32)
    qkv_b = np.asarray(qkv_b, np.float32)
    out_w = np.asarray(out_w, np.float32)
    out_b = np.asarray(out_b, np.float32)

    pi, rank, kr, a, win = _geometry(routes)
    SCALE = 1.0 / float(np.sqrt(DH))

    # masks [QT, NT*win] additive bf16, shared by all cores
    mask_np = np.full((NT, QT, win), -30000.0, np.float32)
    rows = np.repeat(np.arange(QT), K_NEI)
    for t in range(NT):
        krt = (kr[t * QT:(t + 1) * QT] - a[t] * 128).ravel()
        mask_np[t, rows, krt] = 0.0
    mask_np = np.ascontiguousarray(
        mask_np.transpose(1, 0, 2).reshape(QT, NT * win)).astype(ml_dtypes.bfloat16)

    # per-batch permuted transposed activations
    xT_b = [np.ascontiguousarray(x[b][pi].T) for b in range(B)]

    in_maps = []
    for c in range(N_CORES):
        b = c // (N_CORES // B)
        hb = c % (N_CORES // B)
        heads = range(hb * HPC, (hb + 1) * HPC)
        w_rows = []
        b_rows = []
        for sect, scale in ((0, SCALE), (1, 1.0), (2, 1.0)):
            for h in heads:
                r0 = sect * DIM + h * DH
                w_rows.append(qkv_w[r0:r0 + DH] * scale)
                b_rows.append(qkv_b[r0:r0 + DH] * scale)
        wq_c = np.ascontiguousarray(np.concatenate(w_rows, 0).T)          # [DIM, 768]
        bq_c = np.concatenate(b_rows, 0).reshape(-1, 1).astype(np.float32)
        wo_c = np.ascontiguousarray(out_w[:, hb * HPC * DH:(hb + 1) * HPC * DH].T)
        in_maps.append({
            "xT": xT_b[b],
            "wq": wq_c,
            "bq": bq_c,
            "wo": wo_c,
            "mask": mask_np,
        })

    nc = _get_module(a, win)
    res = run_bass_kernel_spmd(nc, in_maps, core_ids=list(range(N_CORES)))

    out = np.empty((B, S, DIM), np.float32)
    for b in range(B):
        cores = [c for c in range(N_CORES) if c // (N_CORES // B) == b]
        outT = res.results[cores[0]]["outp"].astype(np.float32)
        for c in cores[1:]:
            outT = outT + res.results[c]["outp"]
        rows_sorted = outT.T                      # [S, DIM] in rank order
        tmp = np.empty_like(rows_sorted)
        tmp[pi] = rows_sorted
        out[b] = tmp + out_b[None, :]
    return out


# revision 8
# speedup vs baseline: 1.0512x; 1.0512x over previous
"""Trainium2 Bass kernel for CantorAttention.

Strategy
--------
The Cantor routes are a pure function of the (quantized) Cantor value of each
position: sorting positions by that value makes every query's 64-key route set
live inside a narrow (<=385-wide) window of the sorted order.  Sparse
attention therefore becomes dense *banded* attention after a host-side
permutation:

  host:   pi = argsort(cantor_val), permute x rows, transpose; build per
          128-query-tile 128-aligned windows of width 384 plus an additive
          bf16 mask (-30000 at non-selected slots).
  device: qkvT projection (fp32r matmuls), banded scores + mask (PE),
          exp+rowsum (ACT, fused accum), normalize (GPSIMD), PE-transpose of
          the probabilities into per-128-chunk column-major buffers, PV
          matmuls accumulating transposed attention output, and the output
          projection producing a partial (4-head) outT block.
  host:   sum the 4 partial outT blocks per batch, transpose, un-permute,
          add the output bias.

Sharding: batch x head-block -> 8 cores (core c: b = c//4, heads 4*(c%4)..).
"""

import sys

sys.path.insert(0, "/opt/trn_rl_repo")

import numpy as np

B, S, DIM = 2, 2048, 1024
HEADS, DH = 16, 64
K_NEI = 64
N_CORES = 8
HPC = 4            # heads per core
QT = 128           # query tile (rows per tile)
NT = S // QT       # 16 query tiles
SUP = 4            # query tiles per supertile (PV batch of 512 queries)
NSUP = NT // SUP

_CACHE = {}


def _cantor_val(seq_len, depth=8):
    pos = np.arange(seq_len, dtype=np.float64)
    x = pos / max(1, seq_len - 1)
    x = np.clip(x, 1e-6, 1.0 - 1e-6)
    val = np.zeros_like(x)
    factor = 0.5
    for _ in range(depth):
        xs = x * 3.0
        digit = np.floor(xs)
        x = xs - digit
        val = val + (digit == 2.0).astype(np.float64) * factor
        factor *= 0.5
    return np.clip(val, 0.0, 1.0)


def _geometry(routes):
    """Window geometry from the runtime routes array."""
    val = _cantor_val(S)
    pi = np.argsort(val, kind="stable").astype(np.int64)
    rank = np.empty(S, np.int64)
    rank[pi] = np.arange(S)
    kr = rank[np.asarray(routes, np.int64)][pi]      # [S, K] key ranks, query-rank order
    lo = kr.min(1)
    hi = kr.max(1) + 1
    for win in (384, 512):
        a = np.zeros(NT, np.int64)
        ok = True
        for t in range(NT):
            l = int(lo[t * QT:(t + 1) * QT].min())
            h = int(hi[t * QT:(t + 1) * QT].max())
            a[t] = min(l // 128, (S - win) // 128)
            if h > a[t] * 128 + win:
                ok = False
                break
        if ok:
            return pi, rank, kr, a, win
    raise ValueError("routes structure incompatible with banded-window kernel")


def _build_module(a, win, loop_n=1, phases="ACD", cheat_dma=False):
    from contextlib import nullcontext

    from concourse import bacc, tile, mybir
    from concourse.masks import make_identity

    f32 = mybir.dt.float32
    f32r = mybir.dt.float32r
    bf16 = mybir.dt.bfloat16
    AF = mybir.ActivationFunctionType
    NCH = win // 128                      # chunks per window
    a = [int(v) for v in a]

    # chunk -> [first tile, last tile] using it
    chunk_tiles = {}
    for t in range(NT):
        for j in range(NCH):
            c = a[t] + j
            lo_t, hi_t = chunk_tiles.get(c, (t, t))
            chunk_tiles[c] = (min(lo_t, t), max(hi_t, t))

    nc = bacc.Bacc("TRN2", target_bir_lowering=False, debug=False)
    xT = nc.dram_tensor("xT", [DIM, S], f32r, kind="ExternalInput").ap()
    wq = nc.dram_tensor("wq", [DIM, 3 * HPC * DH], f32r, kind="ExternalInput").ap()
    bq = nc.dram_tensor("bq", [3 * HPC * DH, 1], f32, kind="ExternalInput").ap()
    wo = nc.dram_tensor("wo", [HPC * DH, DIM], f32r, kind="ExternalInput").ap()
    mask = nc.dram_tensor("mask", [QT, NT * win], bf16, kind="ExternalInput").ap()
    outp = nc.dram_tensor("outp", [DIM, S], f32, kind="ExternalOutput").ap()

    NQKV = 3 * HPC * DH                  # 768 rows of qkvT
    NMT = NQKV // 128                    # 6 row-tiles of qkvT

    with tile.TileContext(nc) as tc:
        with tc.tile_pool(name="persist", bufs=1) as pp:
            id32 = pp.tile([128, 128], f32)
            make_identity(nc, id32)
            id_r = pp.tile([128, 128], f32r)
            nc.vector.tensor_copy(id_r, id32)
            id_b = pp.tile([128, 128], bf16)
            nc.vector.tensor_copy(id_b, id32)
            mask_sb = pp.tile([QT, NT * win], bf16)
            nc.sync.dma_start(out=mask_sb, in_=mask)
            bq_sb = []
            for m in range(NMT):
                bt = pp.tile([128, 1], f32, tag=f"bq{m}", name=f"bq{m}")
                nc.sync.dma_start(out=bt, in_=bq[m * 128:(m + 1) * 128, :])
                bq_sb.append(bt)
            qkvT = [pp.tile([128, S], f32r, tag=f"qkvT{m}", name=f"qkvT{m}")
                    for m in range(NMT)]
            attn_outT = [pp.tile([128, S], f32r, tag=f"aout{p}", name=f"aout{p}")
                         for p in range(2)]
            wo_sb = []
            for p2 in range(2):
                wt = pp.tile([128, DIM], f32r, tag=f"wo{p2}", name=f"wo{p2}")
                nc.sync.dma_start(out=wt, in_=wo[p2 * 128:(p2 + 1) * 128, :])
                wo_sb.append(wt)

            loop_cm = tc.For_i(0, loop_n, 1) if loop_n > 1 else nullcontext()
            with loop_cm:
                # ------------- Phase A: qkvT = wq.T @ xT (+bias) -------------
                if "A" in phases:
                    with tc.tile_pool(name="phA", bufs=1) as pa, \
                         tc.tile_pool(name="phAx", bufs=2) as pax, \
                         tc.tile_pool(name="psA", bufs=3, space="PSUM") as psa:
                        wq_sb = []
                        for kk in range(8):
                            wt = pa.tile([128, NQKV], f32r, tag=f"wq{kk}",
                                         name=f"wq{kk}")
                            nc.sync.dma_start(out=wt, in_=wq[kk * 128:(kk + 1) * 128, :])
                            wq_sb.append(wt)
                        xt_prev = None
                        for n in range(4):
                            if cheat_dma and n > 0:
                                xt = xt_prev
                            else:
                                xt = []
                                for kk in range(8):
                                    t_ = pax.tile([128, 512], f32r, tag=f"x{kk}",
                                                  name=f"x{kk}_{n}")
                                    nc.sync.dma_start(
                                        out=t_,
                                        in_=xT[kk * 128:(kk + 1) * 128,
                                               n * 512:(n + 1) * 512])
                                    xt.append(t_)
                                xt_prev = xt
                            for m in range(NMT):
                                ps = psa.tile([128, 512], f32, tag="ps")
                                for kk in range(8):
                                    nc.tensor.matmul(
                                        ps, wq_sb[kk][:, m * 128:(m + 1) * 128], xt[kk],
                                        start=(kk == 0), stop=(kk == 7))
                                nc.vector.tensor_scalar_add(
                                    qkvT[m][:, n * 512:(n + 1) * 512], ps, bq_sb[m])

                # ---------- Phases B+C: V transpose + banded attention ----------
                if "C" in phases:
                    with tc.tile_pool(name="phC", bufs=1) as pc, \
                         tc.tile_pool(name="pexp_pool", bufs=6) as pe_pool, \
                         tc.tile_pool(name="pt_pool", bufs=10) as pt_pool, \
                         tc.tile_pool(name="small", bufs=16) as sm_pool, \
                         tc.tile_pool(name="psB", bufs=3, space="PSUM") as psb, \
                         tc.tile_pool(name="psS", bufs=3, space="PSUM") as pss, \
                         tc.tile_pool(name="psO", bufs=2, space="PSUM") as pso:
                        V_sb = [pc.tile([128, 2 * 128], f32r, tag=f"V{cc}",
                                        name=f"V{cc}") for cc in range(NT)]
                        for cc in range(NT):
                            for s_ in range(2):
                                pv = psb.tile([128, 128], f32r, tag="ptr")
                                nc.tensor.transpose(
                                    pv, qkvT[4 + s_][:, cc * 128:(cc + 1) * 128], id_r)
                                dst = V_sb[cc][:, s_ * 128:(s_ + 1) * 128]
                                if (cc + s_) % 2 == 0:
                                    nc.vector.tensor_copy(dst, pv)
                                else:
                                    nc.scalar.copy(dst, pv)

                        aoutB = [pc.tile([64, S], f32r, tag=f"aoutB{i}",
                                         name=f"aoutB{i}") for i in range(2)]
                        for h in range(HPC):
                            poff = (h % 2) * 64
                            qTh = qkvT[h // 2]
                            kTh = qkvT[2 + h // 2]
                            pt_tiles = {}
                            for u in range(NSUP):
                                for t in range(u * SUP, (u + 1) * SUP):
                                    w0 = a[t] * 128
                                    ps_s = pss.tile([128, win], f32, tag="sc")
                                    nc.tensor.matmul(
                                        ps_s,
                                        qTh[poff:poff + 64, t * 128:(t + 1) * 128],
                                        kTh[poff:poff + 64, w0:w0 + win],
                                        start=True, stop=False, skip_group_check=True)
                                    nc.tensor.matmul(
                                        ps_s, id_b, mask_sb[:, t * win:(t + 1) * win],
                                        start=False, stop=True, skip_group_check=True)
                                    pexp = pe_pool.tile([128, win], f32, tag="pexp")
                                    denom = sm_pool.tile([128, 1], f32, tag="den")
                                    nc.scalar.activation(out=pexp, in_=ps_s,
                                                         func=AF.Exp, accum_out=denom)
                                    recip = sm_pool.tile([128, 1], f32, tag="rec")
                                    nc.vector.reciprocal(recip, denom)
                                    pnorm = pe_pool.tile([128, win], f32r, tag="pnorm")
                                    nc.gpsimd.tensor_scalar_mul(pnorm, pexp, recip)
                                    for j in range(NCH):
                                        c = a[t] + j
                                        t0c, t1c = chunk_tiles[c]
                                        if c not in pt_tiles:
                                            pt_tiles[c] = pt_pool.tile(
                                                [128, (t1c - t0c + 1) * 128], f32r,
                                                tag="pt", name=f"pt_h{h}_c{c}")
                                        ptp = psb.tile([128, 128], f32r, tag="ptr")
                                        nc.tensor.transpose(
                                            ptp, pnorm[:, j * 128:(j + 1) * 128], id_r)
                                        nc.vector.tensor_copy(
                                            pt_tiles[c][:, (t - t0c) * 128:
                                                        (t - t0c + 1) * 128], ptp)
                                # PV for supertile u: pieces (chunk, o0, o1);
                                # widest chunk first with start=True, straddlers
                                # split at the written watermark
                                chunks_u = sorted({a[t] + j
                                                   for t in range(u * SUP,
                                                                  (u + 1) * SUP)
                                                   for j in range(NCH)})
                                ranges = []
                                for c in chunks_u:
                                    t0c, t1c = chunk_tiles[c]
                                    tlo = max(t0c, u * SUP)
                                    thi = min(t1c, (u + 1) * SUP - 1)
                                    ranges.append((c, tlo * 128 - u * 512,
                                                   (thi + 1) * 128 - u * 512))
                                first = max(ranges, key=lambda r: r[2] - r[1])
                                pieces = [first]
                                wlo, whi = first[1], first[2]
                                for c, o0, o1 in sorted(
                                        (r for r in ranges if r is not first),
                                        key=lambda r: r[1]):
                                    for p0, p1 in ((o0, min(o1, wlo)),
                                                   (max(o0, wlo), min(o1, whi)),
                                                   (max(o0, whi), o1)):
                                        if p1 > p0:
                                            pieces.append((c, p0, p1))
                                    wlo, whi = min(wlo, o0), max(whi, o1)
                                po = pso.tile([128, 512], f32, tag="po")
                                for i_p, (c, o0, o1) in enumerate(pieces):
                                    t0c, _ = chunk_tiles[c]
                                    r0 = o0 + u * 512 - t0c * 128
                                    r1 = o1 + u * 512 - t0c * 128
                                    nc.tensor.matmul(
                                        po[0:64, o0:o1],
                                        V_sb[c][:, h * 64:(h + 1) * 64],
                                        pt_tiles[c][:, r0:r1],
                                        start=(i_p == 0),
                                        stop=(i_p == len(pieces) - 1),
                                        skip_group_check=True)
                                if poff == 0:
                                    dst = attn_outT[h // 2][0:64,
                                                            u * 512:(u + 1) * 512]
                                else:
                                    dst = aoutB[h // 2][:, u * 512:(u + 1) * 512]
                                nc.vector.tensor_copy(dst, po[0:64, :])
                            if poff != 0:
                                nc.sync.dma_start(out=attn_outT[h // 2][64:128, :],
                                                  in_=aoutB[h // 2])

                # ------------- Phase D: outp = wo.T @ attn_outT -------------
                if "D" in phases:
                    with tc.tile_pool(name="phD", bufs=2) as pd, \
                         tc.tile_pool(name="psD", bufs=2, space="PSUM") as psd:
                        for mm in range(8):
                            st = pd.tile([128, S], f32, tag="st")
                            for n in range(4):
                                ps = psd.tile([128, 512], f32, tag="pod")
                                for p2 in range(2):
                                    nc.tensor.matmul(
                                        ps, wo_sb[p2][:, mm * 128:(mm + 1) * 128],
                                        attn_outT[p2][:, n * 512:(n + 1) * 512],
                                        start=(p2 == 0), stop=(p2 == 1))
                                if (mm + n) % 2 == 0:
                                    nc.scalar.copy(st[:, n * 512:(n + 1) * 512], ps)
                                else:
                                    nc.vector.tensor_copy(st[:, n * 512:(n + 1) * 512],
                                                          ps)
                            nc.sync.dma_start(out=outp[mm * 128:(mm + 1) * 128, :],
                                              in_=st)

    nc.compile()
    return nc


def _get_module(a, win):
    key = (tuple(int(v) for v in a), int(win))
    if key not in _CACHE:
        _CACHE[key] = _build_module(a, win)
    return _CACHE[key]


def kernel(x, routes, qkv_w, qkv_b, out_w, out_b):
    import ml_dtypes
    from concourse.bass_utils import run_bass_kernel_spmd

    x = np.ascontiguousarray(np.asarray(x, np.float32))
    routes = np.asarray(routes)
    qkv_w = np.asarray(qkv_w, np.float32)
    qkv_b = np.asarray(qkv_b, np.float32)
    out_w = np.asarray(out_w, np.float32)
    out_b = np.asarray(out_b, np.float32)

    pi, rank, kr, a, win = _geometry(routes)
    SCALE = 1.0 / float(np.sqrt(DH))

    # masks [QT, NT*win] additive bf16, shared by all cores
    mask_np = np.full((NT, QT, win), -30000.0, np.float32)
    rows = np.repeat(np.arange(QT), K_NEI)
    for t in range(NT):
        krt = (kr[t * QT:(t + 1) * QT] - a[t] * 128).ravel()
        mask_np[t, rows, krt] = 0.0
    mask_np = np.ascontiguousarray(
        mask_np.transpose(1, 0, 2).reshape(QT, NT * win)).astype(ml_dtypes.bfloat16)

    xT_b = [np.ascontiguousarray(x[b][pi].T) for b in range(B)]

    in_maps = []
    for c in range(N_CORES):
        b = c // (N_CORES // B)
        hb = c % (N_CORES // B)
        heads = range(hb * HPC, (hb + 1) * HPC)
        w_rows = []
        b_rows = []
        for sect, scale in ((0, SCALE), (1, 1.0), (2, 1.0)):
            for h in heads:
                r0 = sect * DIM + h * DH
                w_rows.append(qkv_w[r0:r0 + DH] * scale)
                b_rows.append(qkv_b[r0:r0 + DH] * scale)
        wq_c = np.ascontiguousarray(np.concatenate(w_rows, 0).T)          # [DIM, 768]
        bq_c = np.concatenate(b_rows, 0).reshape(-1, 1).astype(np.float32)
        wo_c = np.ascontiguousarray(out_w[:, hb * HPC * DH:(hb + 1) * HPC * DH].T)
        in_maps.append({
            "xT": xT_b[b],
            "wq": wq_c,
            "bq": bq_c,
            "wo": wo_c,
            "mask": mask_np,
        })

    nc = _get_module(a, win)
    res = run_bass_kernel_spmd(nc, in_maps, core_ids=list(range(N_CORES)))

    out = np.empty((B, S, DIM), np.float32)
    for b in range(B):
        cores = [c for c in range(N_CORES) if c // (N_CORES // B) == b]
        outT = res.results[cores[0]]["outp"].astype(np.float32)
        for c in cores[1:]:
            outT = outT + res.results[c]["outp"]
        rows_sorted = outT.T                      # [S, DIM] in rank order
        tmp = np.empty_like(rows_sorted)
        tmp[pi] = rows_sorted
        out[b] = tmp + out_b[None, :]
    return out
